# revision 6
# baseline (speedup 1.0000x reference)
"""MoE top-2 (2 experts) FFN kernel for TRN2, 8 NeuronCores.

Problem (hardcoded):
  x:   (8192, 2048) f32 tokens
  two expert FFNs: d_model=2048 -> d_ff=8192 (gelu exact) -> 2048
  out[i] = w0[i] * FFN0(x[i]) + w1[i] * FFN1(x[i])
  where w_e[i] = sum of top2_weight[i, s] over slots s with (top2_exp_id[i,s] % 2) == e

Strategy:
  - Host: fold top-2 gating into two per-token scalars w0/w1; transpose x.
  - Data-parallel over tokens: each of 8 cores takes 1024 tokens and runs
    both experts densely (every token visits both experts, weighted).
  - On-core: activations kept transposed ([d_model|d_ff on partitions] x
    [tokens on free dim]) so both matmul layers contract along partitions
    with weights in their natural HBM layout.
  - fp32r (FP22) matmuls at moving-dim 512 -> full 1-cycle/row PE speed.
  - d_ff processed in chunks of 512; layer-2 partials accumulated into an
    SBUF-resident y to avoid reloading weights (each W is streamed once).
"""

import os

import numpy as np

import concourse.bass as bass
import concourse.mybir as mybir
import concourse.tile as tile
from concourse import bacc
from concourse import bass_utils


def _ensure_ntff_hook():
    """This image's `antenv` lacks `axon_hooks`, so boot-time NTFF hook
    install degrades silently and trace=True captures nothing. Register a
    shim module and install the ctypes-driven hook (same as trn_boot)."""
    import sys
    import types

    if "antenv.axon_hooks" in sys.modules:
        return
    mod = types.ModuleType("antenv.axon_hooks")
    mod._hook = None

    def set_axon_ntff_profile_hook(h):
        mod._hook = h

    def get_axon_ntff_profile_hook():
        return mod._hook

    mod.set_axon_ntff_profile_hook = set_axon_ntff_profile_hook
    mod.get_axon_ntff_profile_hook = get_axon_ntff_profile_hook
    sys.modules["antenv.axon_hooks"] = mod
    try:
        from trn_agent_boot.trn_boot import _ntff_profile_via_ctypes

        hook = _ntff_profile_via_ctypes("/opt/axon/libaxon_pjrt.so")
        if hook is not None:
            mod._hook = hook
    except Exception:
        pass

P = 128
D_MODEL = 2048
D_FF = 8192
N_LOCAL = 8192
N_CORES = 8
TOKC = N_LOCAL // N_CORES      # 1024 tokens per core
HALF = 512                     # matmul moving free dim (fp32 max)
NH = TOKC // HALF              # 2
KM = D_MODEL // P              # 16 contraction tiles for layer 1
CHUNK = 512                    # d_ff chunk held in PSUM per pass
FC = CHUNK // P                # 4 d_ff tiles per chunk
NCHUNK = D_FF // CHUNK         # 16
M2 = D_MODEL // P              # 16 output d_model tiles

F32 = mybir.dt.float32
F32R = mybir.dt.float32r
GELU = mybir.ActivationFunctionType.Gelu


def _build(nc):
    xt = nc.dram_tensor("xt", (D_MODEL, TOKC), F32R, kind="ExternalInput").ap()
    w1 = [
        nc.dram_tensor(f"w1_{e}", (D_MODEL, D_FF), F32R, kind="ExternalInput").ap()
        for e in range(2)
    ]
    w2 = [
        nc.dram_tensor(f"w2_{e}", (D_FF, D_MODEL), F32R, kind="ExternalInput").ap()
        for e in range(2)
    ]
    b1t = [
        nc.dram_tensor(f"b1t_{e}", (P, D_FF // P), F32, kind="ExternalInput").ap()
        for e in range(2)
    ]
    b2t = [
        nc.dram_tensor(f"b2t_{e}", (P, M2), F32, kind="ExternalInput").ap()
        for e in range(2)
    ]
    wg = [
        nc.dram_tensor(f"wg{e}", (P, TOKC), F32, kind="ExternalInput").ap()
        for e in range(2)
    ]
    yt = nc.dram_tensor("yt", (D_MODEL, TOKC), F32, kind="ExternalOutput").ap()

    with tile.TileContext(nc) as tc:
        with (
            tc.tile_pool(name="const", bufs=1) as const_pool,
            tc.tile_pool(name="w1s", bufs=4) as w1_pool,
            tc.tile_pool(name="w2s", bufs=4) as w2_pool,
            tc.tile_pool(name="ht", bufs=5) as ht_pool,
            tc.tile_pool(name="ps", bufs=8, space="PSUM") as psum_pool,
        ):
            # ---- persistent SBUF state ----
            xt_sb = const_pool.tile([P, KM, TOKC], F32R, tag="xt", name="xt_sb")
            y_sb = const_pool.tile([P, M2, TOKC], F32, tag="y", name="y_sb")
            wg_sb = [
                const_pool.tile([P, TOKC], F32, tag=f"wg{e}", name=f"wg{e}_sb")
                for e in range(2)
            ]
            b1t_sb = [
                const_pool.tile([P, D_FF // P], F32, tag=f"b1t{e}", name=f"b1t{e}_sb")
                for e in range(2)
            ]
            b2t_sb = [
                const_pool.tile([P, M2], F32, tag=f"b2t{e}", name=f"b2t{e}_sb")
                for e in range(2)
            ]

            xt3 = xt.rearrange("(ko p) t -> p ko t", p=P)
            for k in range(KM):
                nc.sync.dma_start(xt_sb[:, k, :], xt3[:, k, :])
            for e in range(2):
                nc.sync.dma_start(wg_sb[e][:], wg[e][:])
                nc.sync.dma_start(b1t_sb[e][:], b1t[e][:])
                nc.sync.dma_start(b2t_sb[e][:], b2t[e][:])

            # ---- y init: bias-2 term, gated: y = wg0*b2_0[d] + wg1*b2_1[d] ----
            for m in range(M2):
                nc.vector.tensor_scalar_mul(
                    y_sb[:, m, :], wg_sb[0][:], b2t_sb[0][:, m : m + 1]
                )
                t = ht_pool.tile([P, TOKC], F32, tag="ht", name="ytmp")
                nc.vector.tensor_scalar_mul(
                    t[:], wg_sb[1][:], b2t_sb[1][:, m : m + 1]
                )
                nc.vector.tensor_add(y_sb[:, m, :], y_sb[:, m, :], t[:])

            # ---- main loop: experts x d_ff chunks ----
            for e in range(2):
                for c in range(NCHUNK):
                    # layer 1: h_pre[f, h] in PSUM, contraction over d_model
                    psums = [
                        [
                            psum_pool.tile(
                                [P, HALF], F32, tag="ps", name=f"ps1_{e}_{c}_{f}_{h}"
                            )
                            for h in range(NH)
                        ]
                        for f in range(FC)
                    ]
                    for k in range(KM):
                        w1s = w1_pool.tile(
                            [P, CHUNK], F32R, tag="w1s", name=f"w1s_{e}_{c}_{k}"
                        )
                        nc.sync.dma_start(
                            w1s[:],
                            w1[e][k * P : (k + 1) * P, c * CHUNK : (c + 1) * CHUNK],
                        )
                        w1r = w1s
                        for f in range(FC):
                            for h in range(NH):
                                nc.tensor.matmul(
                                    psums[f][h][:],
                                    w1r[:, f * P : (f + 1) * P],
                                    xt_sb[:, k, h * HALF : (h + 1) * HALF],
                                    start=(k == 0),
                                    stop=(k == KM - 1),
                                )

                    # gelu(+bias1) then scale by per-token gate weight
                    hts = []
                    for f in range(FC):
                        ht = ht_pool.tile([P, TOKC], F32R, tag="ht", name=f"ht_{e}_{c}_{f}")
                        col = c * FC + f
                        for h in range(NH):
                            nc.scalar.activation(
                                ht[:, h * HALF : (h + 1) * HALF],
                                psums[f][h][:],
                                GELU,
                                bias=b1t_sb[e][:, col : col + 1],
                            )
                        nc.vector.tensor_mul(ht[:], ht[:], wg_sb[e][:])
                        hts.append(ht)

                    # layer 2: contraction over this chunk's d_ff rows
                    w2s = []
                    for f in range(FC):
                        w2f = w2_pool.tile(
                            [P, D_MODEL], F32R, tag="w2s", name=f"w2s_{e}_{c}_{f}"
                        )
                        row = (c * FC + f) * P
                        nc.sync.dma_start(w2f[:], w2[e][row : row + P, :])
                        w2s.append(w2f)
                    for m in range(M2):
                        for h in range(NH):
                            ps = psum_pool.tile(
                                [P, HALF], F32, tag="ps", name=f"ps2_{e}_{c}_{m}_{h}"
                            )
                            for f in range(FC):
                                nc.tensor.matmul(
                                    ps[:],
                                    w2s[f][:, m * P : (m + 1) * P],
                                    hts[f][:, h * HALF : (h + 1) * HALF],
                                    start=(f == 0),
                                    stop=(f == FC - 1),
                                )
                            ysl = y_sb[:, m, h * HALF : (h + 1) * HALF]
                            nc.vector.tensor_add(ysl, ysl, ps[:])

            # ---- store ----
            yt3 = yt.rearrange("(mo p) t -> p mo t", p=P)
            for m in range(M2):
                nc.sync.dma_start(yt3[:, m, :], y_sb[:, m, :])

    nc.compile()
    return nc


_CACHED_NC = None


def _get_nc():
    global _CACHED_NC
    if _CACHED_NC is None:
        nc = bacc.Bacc(
            "TRN2",
            target_bir_lowering=False,
            debug=False,
            num_devices=N_CORES,
        )
        _CACHED_NC = _build(nc)
    return _CACHED_NC


def kernel(**inputs):
    x = np.asarray(inputs["x_local"], dtype=np.float32)          # (8192, 2048)
    ids = np.asarray(inputs["top2_exp_id"])                       # (8192, 2)
    tw = np.asarray(inputs["top2_weight"], dtype=np.float32)      # (8192, 2)

    sel = (ids % 2).astype(np.float32)
    wg1 = (tw * sel).sum(axis=1).astype(np.float32)               # expert-1 gate
    wg0 = (tw * (1.0 - sel)).sum(axis=1).astype(np.float32)       # expert-0 gate

    xt = np.ascontiguousarray(x.T)                                # (2048, 8192)

    shared = {}
    for e in range(2):
        shared[f"w1_{e}"] = np.ascontiguousarray(
            np.asarray(inputs[f"W1_{e}"], dtype=np.float32)
        )
        shared[f"w2_{e}"] = np.ascontiguousarray(
            np.asarray(inputs[f"W2_{e}"], dtype=np.float32)
        )
        shared[f"b1t_{e}"] = np.ascontiguousarray(
            np.asarray(inputs[f"b1_{e}"], dtype=np.float32).reshape(D_FF // P, P).T
        )
        shared[f"b2t_{e}"] = np.ascontiguousarray(
            np.asarray(inputs[f"b2_{e}"], dtype=np.float32).reshape(M2, P).T
        )

    in_maps = []
    for c in range(N_CORES):
        tok = slice(c * TOKC, (c + 1) * TOKC)
        m = dict(shared)
        m["xt"] = np.ascontiguousarray(xt[:, tok])
        m["wg0"] = np.ascontiguousarray(
            np.broadcast_to(wg0[tok], (P, TOKC)).astype(np.float32)
        )
        m["wg1"] = np.ascontiguousarray(
            np.broadcast_to(wg1[tok], (P, TOKC)).astype(np.float32)
        )
        in_maps.append(m)

    nc = _get_nc()
    trace = bool(int(os.environ.get("KERNEL_TRACE", "0")))
    if trace:
        _ensure_ntff_hook()
    res = bass_utils.run_bass_kernel_spmd(
        nc, in_maps, core_ids=list(range(N_CORES)), trace=trace
    )
    if trace:
        kernel.last_exec_time_ns = res.exec_time_ns
        kernel.last_results = res

    yt = np.concatenate([r["yt"] for r in res.results], axis=1)   # (2048, 8192)
    return np.ascontiguousarray(yt.T)                             # (8192, 2048)


# revision 10
# speedup vs baseline: 1.0468x; 1.0468x over previous
"""MoE top-2 (2 experts) FFN kernel for TRN2, 8 NeuronCores.

Problem (hardcoded):
  x:   (8192, 2048) f32 tokens
  two expert FFNs: d_model=2048 -> d_ff=8192 (gelu exact) -> 2048
  out[i] = w0[i] * FFN0(x[i]) + w1[i] * FFN1(x[i])
  where w_e[i] = sum of top2_weight[i, s] over slots s with (top2_exp_id[i,s] % 2) == e

Strategy:
  - Host: fold top-2 gating into two per-token scalars w0/w1; transpose x.
  - Data-parallel over tokens: each of 8 cores takes 1024 tokens and runs
    both experts densely (every token visits both experts, weighted).
  - On-core: activations kept transposed ([d_model|d_ff on partitions] x
    [tokens on free dim]) so both matmul layers contract along partitions
    with weights in their natural HBM layout.
  - fp32r (FP22) matmuls at moving-dim 512 -> full 1-cycle/row PE speed.
  - d_ff processed in chunks of 512; layer-2 partials accumulated into an
    SBUF-resident y to avoid reloading weights (each W is streamed once).
"""

import os

import numpy as np

import concourse.bass as bass
import concourse.mybir as mybir
import concourse.tile as tile
from concourse import bacc
from concourse import bass_utils


def _ensure_ntff_hook():
    """This image's `antenv` lacks `axon_hooks`, so boot-time NTFF hook
    install degrades silently and trace=True captures nothing. Register a
    shim module and install the ctypes-driven hook (same as trn_boot)."""
    import sys
    import types

    if "antenv.axon_hooks" in sys.modules:
        return
    mod = types.ModuleType("antenv.axon_hooks")
    mod._hook = None

    def set_axon_ntff_profile_hook(h):
        mod._hook = h

    def get_axon_ntff_profile_hook():
        return mod._hook

    mod.set_axon_ntff_profile_hook = set_axon_ntff_profile_hook
    mod.get_axon_ntff_profile_hook = get_axon_ntff_profile_hook
    sys.modules["antenv.axon_hooks"] = mod
    try:
        from trn_agent_boot.trn_boot import _ntff_profile_via_ctypes

        hook = _ntff_profile_via_ctypes("/opt/axon/libaxon_pjrt.so")
        if hook is not None:
            mod._hook = hook
    except Exception:
        pass

P = 128
D_MODEL = 2048
D_FF = 8192
N_LOCAL = 8192
N_CORES = 8
TOKC = N_LOCAL // N_CORES      # 1024 tokens per core
HALF = 512                     # matmul moving free dim (fp32 max)
NH = TOKC // HALF              # 2
KM = D_MODEL // P              # 16 contraction tiles for layer 1
CHUNK = 512                    # d_ff chunk held in PSUM per pass
FC = CHUNK // P                # 4 d_ff tiles per chunk
NCHUNK = D_FF // CHUNK         # 16
M2 = D_MODEL // P              # 16 output d_model tiles

F32 = mybir.dt.float32
F32R = mybir.dt.float32r
GELU = mybir.ActivationFunctionType.Gelu


def _build(nc):
    xt = nc.dram_tensor("xt", (D_MODEL, TOKC), F32R, kind="ExternalInput").ap()
    w1 = [
        nc.dram_tensor(f"w1_{e}", (D_MODEL, D_FF), F32R, kind="ExternalInput").ap()
        for e in range(2)
    ]
    w2 = [
        nc.dram_tensor(f"w2_{e}", (D_FF, D_MODEL), F32R, kind="ExternalInput").ap()
        for e in range(2)
    ]
    b1t = [
        nc.dram_tensor(f"b1t_{e}", (P, D_FF // P), F32, kind="ExternalInput").ap()
        for e in range(2)
    ]
    b2t = [
        nc.dram_tensor(f"b2t_{e}", (P, M2), F32, kind="ExternalInput").ap()
        for e in range(2)
    ]
    wg = [
        nc.dram_tensor(f"wg{e}", (P, TOKC), F32, kind="ExternalInput").ap()
        for e in range(2)
    ]
    yt = nc.dram_tensor("yt", (D_MODEL, TOKC), F32, kind="ExternalOutput").ap()

    with tile.TileContext(nc) as tc:
        with (
            tc.tile_pool(name="const", bufs=1) as const_pool,
            tc.tile_pool(name="w1s", bufs=5) as w1_pool,
            tc.tile_pool(name="w2s", bufs=5) as w2_pool,
            tc.tile_pool(name="ht", bufs=5) as ht_pool,
            tc.tile_pool(name="ps", bufs=8, space="PSUM") as psum_pool,
        ):
            # ---- persistent SBUF state ----
            xt_sb = const_pool.tile([P, KM, TOKC], F32R, tag="xt", name="xt_sb")
            y_sb = const_pool.tile([P, M2, TOKC], F32, tag="y", name="y_sb")
            wg_sb = [
                const_pool.tile([P, TOKC], F32, tag=f"wg{e}", name=f"wg{e}_sb")
                for e in range(2)
            ]
            b1t_sb = [
                const_pool.tile([P, D_FF // P], F32, tag=f"b1t{e}", name=f"b1t{e}_sb")
                for e in range(2)
            ]
            b2t_sb = [
                const_pool.tile([P, M2], F32, tag=f"b2t{e}", name=f"b2t{e}_sb")
                for e in range(2)
            ]

            xt3 = xt.rearrange("(ko p) t -> p ko t", p=P)

            pairs = [(e, c) for e in range(2) for c in range(NCHUNK)]

            def emit_l1(e, c, first=False):
                """PE: layer-1 matmuls for one (expert, chunk). Returns psums."""
                psums = [
                    [
                        psum_pool.tile(
                            [P, HALF], F32, tag="ps", name=f"ps1_{e}_{c}_{f}_{h}"
                        )
                        for h in range(NH)
                    ]
                    for f in range(FC)
                ]
                for k in range(KM):
                    if first:
                        # interleave resident-input loads with the first
                        # chunk's weight strips so PE can start immediately
                        nc.sync.dma_start(xt_sb[:, k, :], xt3[:, k, :])
                        if k == 0:
                            for ee in range(2):
                                nc.sync.dma_start(wg_sb[ee][:], wg[ee][:])
                                nc.sync.dma_start(b1t_sb[ee][:], b1t[ee][:])
                                nc.sync.dma_start(b2t_sb[ee][:], b2t[ee][:])
                    w1s = w1_pool.tile(
                        [P, CHUNK], F32R, tag="w1s", name=f"w1s_{e}_{c}_{k}"
                    )
                    nc.sync.dma_start(
                        w1s[:],
                        w1[e][k * P : (k + 1) * P, c * CHUNK : (c + 1) * CHUNK],
                    )
                    for f in range(FC):
                        for h in range(NH):
                            nc.tensor.matmul(
                                psums[f][h][:],
                                w1s[:, f * P : (f + 1) * P],
                                xt_sb[:, k, h * HALF : (h + 1) * HALF],
                                start=(k == 0),
                                stop=(k == KM - 1),
                            )
                return psums

            def emit_act(e, c, psums):
                """ACT+DVE: gelu(+b1), gate scale. Also W2 strip loads."""
                hts = []
                for f in range(FC):
                    ht = ht_pool.tile(
                        [P, TOKC], F32R, tag="ht", name=f"ht_{e}_{c}_{f}"
                    )
                    col = c * FC + f
                    for h in range(NH):
                        nc.scalar.activation(
                            ht[:, h * HALF : (h + 1) * HALF],
                            psums[f][h][:],
                            GELU,
                            bias=b1t_sb[e][:, col : col + 1],
                        )
                    nc.vector.tensor_mul(ht[:], ht[:], wg_sb[e][:])
                    hts.append(ht)
                w2s = []
                for f in range(FC):
                    w2f = w2_pool.tile(
                        [P, D_MODEL], F32R, tag="w2s", name=f"w2s_{e}_{c}_{f}"
                    )
                    row = (c * FC + f) * P
                    nc.sync.dma_start(w2f[:], w2[e][row : row + P, :])
                    w2s.append(w2f)
                return hts, w2s

            def emit_l2(e, c, hts, w2s):
                """PE: layer-2 matmuls; DVE: accumulate into y."""
                for m in range(M2):
                    for h in range(NH):
                        ps = psum_pool.tile(
                            [P, HALF], F32, tag="ps", name=f"ps2_{e}_{c}_{m}_{h}"
                        )
                        for f in range(FC):
                            nc.tensor.matmul(
                                ps[:],
                                w2s[f][:, m * P : (m + 1) * P],
                                hts[f][:, h * HALF : (h + 1) * HALF],
                                start=(f == 0),
                                stop=(f == FC - 1),
                            )
                        ysl = y_sb[:, m, h * HALF : (h + 1) * HALF]
                        nc.vector.tensor_add(ysl, ysl, ps[:])

            # software pipeline: PE order is L1(0), L1(1), L2(0), L1(2),
            # L2(1), ... so layer-2 of chunk i runs while ACT/DVE finish
            # h(i) during L1(i+1), leaving no PE stall at chunk bounds.
            psums_cur = emit_l1(*pairs[0], first=True)

            # y init: gated bias-2 term, y = wg0*b2_0[d] + wg1*b2_1[d]
            for m in range(M2):
                nc.vector.tensor_scalar_mul(
                    y_sb[:, m, :], wg_sb[0][:], b2t_sb[0][:, m : m + 1]
                )
                t = ht_pool.tile([P, TOKC], F32, tag="ht", name="ytmp")
                nc.vector.tensor_scalar_mul(
                    t[:], wg_sb[1][:], b2t_sb[1][:, m : m + 1]
                )
                nc.vector.tensor_add(y_sb[:, m, :], y_sb[:, m, :], t[:])

            for i, (e, c) in enumerate(pairs):
                hts, w2s = emit_act(e, c, psums_cur)
                if i + 1 < len(pairs):
                    psums_cur = emit_l1(*pairs[i + 1])
                emit_l2(e, c, hts, w2s)

            # ---- store ----
            yt3 = yt.rearrange("(mo p) t -> p mo t", p=P)
            for m in range(M2):
                nc.sync.dma_start(yt3[:, m, :], y_sb[:, m, :])

    nc.compile()
    return nc


_CACHED_NC = None


def _get_nc():
    global _CACHED_NC
    if _CACHED_NC is None:
        nc = bacc.Bacc(
            "TRN2",
            target_bir_lowering=False,
            debug=False,
            num_devices=N_CORES,
        )
        _CACHED_NC = _build(nc)
    return _CACHED_NC


def kernel(**inputs):
    x = np.asarray(inputs["x_local"], dtype=np.float32)          # (8192, 2048)
    ids = np.asarray(inputs["top2_exp_id"])                       # (8192, 2)
    tw = np.asarray(inputs["top2_weight"], dtype=np.float32)      # (8192, 2)

    sel = (ids % 2).astype(np.float32)
    wg1 = (tw * sel).sum(axis=1).astype(np.float32)               # expert-1 gate
    wg0 = (tw * (1.0 - sel)).sum(axis=1).astype(np.float32)       # expert-0 gate

    xt = np.ascontiguousarray(x.T)                                # (2048, 8192)

    shared = {}
    for e in range(2):
        shared[f"w1_{e}"] = np.ascontiguousarray(
            np.asarray(inputs[f"W1_{e}"], dtype=np.float32)
        )
        shared[f"w2_{e}"] = np.ascontiguousarray(
            np.asarray(inputs[f"W2_{e}"], dtype=np.float32)
        )
        shared[f"b1t_{e}"] = np.ascontiguousarray(
            np.asarray(inputs[f"b1_{e}"], dtype=np.float32).reshape(D_FF // P, P).T
        )
        shared[f"b2t_{e}"] = np.ascontiguousarray(
            np.asarray(inputs[f"b2_{e}"], dtype=np.float32).reshape(M2, P).T
        )

    in_maps = []
    for c in range(N_CORES):
        tok = slice(c * TOKC, (c + 1) * TOKC)
        m = dict(shared)
        m["xt"] = np.ascontiguousarray(xt[:, tok])
        m["wg0"] = np.ascontiguousarray(
            np.broadcast_to(wg0[tok], (P, TOKC)).astype(np.float32)
        )
        m["wg1"] = np.ascontiguousarray(
            np.broadcast_to(wg1[tok], (P, TOKC)).astype(np.float32)
        )
        in_maps.append(m)

    nc = _get_nc()
    trace = bool(int(os.environ.get("KERNEL_TRACE", "0")))
    if trace:
        _ensure_ntff_hook()
    res = bass_utils.run_bass_kernel_spmd(
        nc, in_maps, core_ids=list(range(N_CORES)), trace=trace
    )
    if trace:
        kernel.last_exec_time_ns = res.exec_time_ns
        kernel.last_results = res

    yt = np.concatenate([r["yt"] for r in res.results], axis=1)   # (2048, 8192)
    return np.ascontiguousarray(yt.T)                             # (8192, 2048)


# revision 11
# speedup vs baseline: 1.3036x; 1.2453x over previous
"""MoE top-2 (2 experts) FFN kernel for TRN2, 8 NeuronCores.

Problem (hardcoded):
  x:   (8192, 2048) f32 tokens
  two expert FFNs: d_model=2048 -> d_ff=8192 (gelu exact) -> 2048
  out[i] = w0[i] * FFN0(x[i]) + w1[i] * FFN1(x[i])
  where w_e[i] = sum of top2_weight[i, s] over slots s with (top2_exp_id[i,s] % 2) == e

Strategy:
  - Host: fold top-2 gating into per-token scalars w0/w1; transpose x;
    gather each expert's active tokens (those with w_e > 0, ~75% of
    tokens) into a padded capacity of 832 per core -> 18.75% fewer FLOPs
    than dense. Dense fallback if a gather overflows capacity.
  - Data-parallel over tokens: each of 8 cores takes 1024 tokens.
  - On-core: activations kept transposed ([d_model|d_ff on partitions] x
    [tokens on free dim]) so both matmul layers contract along partitions
    with weights in their natural HBM layout.
  - fp32r (FP22) matmuls with moving dim >= 256 -> full 1-cycle/row PE.
  - d_ff processed in chunks of 512; layer-2 partials accumulated into an
    SBUF-resident y so each weight byte is streamed exactly once.
  - Software-pipelined emission: PE order L1(0),L1(1),L2(0),L1(2),L2(1)...
    so gelu/gate (ACT+DVE) of chunk i overlaps L1(i+1) matmuls.
"""

import os

import numpy as np

import concourse.bass as bass
import concourse.mybir as mybir
import concourse.tile as tile
from concourse import bacc
from concourse import bass_utils


def _ensure_ntff_hook():
    """This image's `antenv` lacks `axon_hooks`, so boot-time NTFF hook
    install degrades silently and trace=True captures nothing. Register a
    shim module and install the ctypes-driven hook (same as trn_boot)."""
    import sys
    import types

    if "antenv.axon_hooks" in sys.modules:
        return
    mod = types.ModuleType("antenv.axon_hooks")
    mod._hook = None

    def set_axon_ntff_profile_hook(h):
        mod._hook = h

    def get_axon_ntff_profile_hook():
        return mod._hook

    mod.set_axon_ntff_profile_hook = set_axon_ntff_profile_hook
    mod.get_axon_ntff_profile_hook = get_axon_ntff_profile_hook
    sys.modules["antenv.axon_hooks"] = mod
    try:
        from trn_agent_boot.trn_boot import _ntff_profile_via_ctypes

        hook = _ntff_profile_via_ctypes("/opt/axon/libaxon_pjrt.so")
        if hook is not None:
            mod._hook = hook
    except Exception:
        pass


P = 128
D_MODEL = 2048
D_FF = 8192
N_LOCAL = 8192
N_CORES = 8
TOKC = N_LOCAL // N_CORES      # 1024 tokens per core
CAP = 832                      # per-expert gathered-token capacity per core
KM = D_MODEL // P              # 16 contraction tiles for layer 1
CHUNK = 512                    # d_ff chunk held in PSUM per pass
FC = CHUNK // P                # 4 d_ff tiles per chunk
NCHUNK = D_FF // CHUNK         # 16
M2 = D_MODEL // P              # 16 output d_model tiles

F32 = mybir.dt.float32
F32R = mybir.dt.float32r
GELU = mybir.ActivationFunctionType.Gelu


def _blocks(total):
    """Moving-dim blocks: each <= 512 (fp32 max) and >= 256 (fp32r full
    speed needs ap_size >= 256)."""
    out = []
    off = 0
    while total - off > 512:
        out.append((off, 512))
        off += 512
    out.append((off, total - off))
    assert out[-1][1] >= 256
    return out


def _build_sparse(nc):
    """Per-expert gathered tokens (CAP per core); expert passes run
    back-to-back reusing the same xt/y SBUF buffers."""
    HS = _blocks(CAP)
    xg = [
        nc.dram_tensor(f"xg{e}", (D_MODEL, CAP), F32R, kind="ExternalInput").ap()
        for e in range(2)
    ]
    w1 = [
        nc.dram_tensor(f"w1_{e}", (D_MODEL, D_FF), F32R, kind="ExternalInput").ap()
        for e in range(2)
    ]
    w2 = [
        nc.dram_tensor(f"w2_{e}", (D_FF, D_MODEL), F32R, kind="ExternalInput").ap()
        for e in range(2)
    ]
    b1t = [
        nc.dram_tensor(f"b1t_{e}", (P, D_FF // P), F32, kind="ExternalInput").ap()
        for e in range(2)
    ]
    b2t = [
        nc.dram_tensor(f"b2t_{e}", (P, M2), F32, kind="ExternalInput").ap()
        for e in range(2)
    ]
    wgg = [
        nc.dram_tensor(f"wgg{e}", (P, CAP), F32, kind="ExternalInput").ap()
        for e in range(2)
    ]
    yt = [
        nc.dram_tensor(f"yt{e}", (D_MODEL, CAP), F32, kind="ExternalOutput").ap()
        for e in range(2)
    ]

    with tile.TileContext(nc) as tc:
        with (
            tc.tile_pool(name="const", bufs=1) as const_pool,
            tc.tile_pool(name="w1s", bufs=6) as w1_pool,
            tc.tile_pool(name="w2s", bufs=5) as w2_pool,
            tc.tile_pool(name="ht", bufs=6) as ht_pool,
            tc.tile_pool(name="ps", bufs=8, space="PSUM") as psum_pool,
        ):
            # xt k-tiles are separate so the second expert's reload can
            # start as soon as the first expert's last read of each tile
            # retires (fine-grained WAR deps).
            xt_sb = [
                const_pool.tile([P, CAP], F32R, tag=f"xt{k}", name=f"xt_sb{k}")
                for k in range(KM)
            ]
            y_sb = const_pool.tile([P, M2, CAP], F32, tag="y", name="y_sb")
            wgg_sb = [
                const_pool.tile([P, CAP], F32, tag=f"wgg{e}", name=f"wgg{e}_sb")
                for e in range(2)
            ]
            b1t_sb = [
                const_pool.tile([P, D_FF // P], F32, tag=f"b1t{e}", name=f"b1t{e}_sb")
                for e in range(2)
            ]
            b2t_sb = [
                const_pool.tile([P, M2], F32, tag=f"b2t{e}", name=f"b2t{e}_sb")
                for e in range(2)
            ]

            xg3 = [xg[e].rearrange("(ko p) t -> p ko t", p=P) for e in range(2)]
            yt3 = [yt[e].rearrange("(mo p) t -> p mo t", p=P) for e in range(2)]

            pairs = [(e, c) for e in range(2) for c in range(NCHUNK)]

            def emit_l1(e, c, first=False):
                """PE: layer-1 matmuls for one (expert, chunk)."""
                psums = [
                    [
                        psum_pool.tile(
                            [P, hs], F32, tag="ps", name=f"ps1_{e}_{c}_{f}_{h}"
                        )
                        for h, (off, hs) in enumerate(HS)
                    ]
                    for f in range(FC)
                ]
                for k in range(KM):
                    if c == 0:
                        # (re)load this expert's gathered xT, interleaved
                        # with the first chunk's weight strips
                        nc.sync.dma_start(xt_sb[k][:], xg3[e][:, k, :])
                        if first and k == 0:
                            for ee in range(2):
                                nc.sync.dma_start(wgg_sb[ee][:], wgg[ee][:])
                                nc.sync.dma_start(b1t_sb[ee][:], b1t[ee][:])
                                nc.sync.dma_start(b2t_sb[ee][:], b2t[ee][:])
                    w1s = w1_pool.tile(
                        [P, CHUNK], F32R, tag="w1s", name=f"w1s_{e}_{c}_{k}"
                    )
                    nc.sync.dma_start(
                        w1s[:],
                        w1[e][k * P : (k + 1) * P, c * CHUNK : (c + 1) * CHUNK],
                    )
                    for f in range(FC):
                        for h, (off, hs) in enumerate(HS):
                            nc.tensor.matmul(
                                psums[f][h][:],
                                w1s[:, f * P : (f + 1) * P],
                                xt_sb[k][:, off : off + hs],
                                start=(k == 0),
                                stop=(k == KM - 1),
                            )
                return psums

            def emit_act(e, c, psums):
                """ACT+DVE: gelu(+b1), gate scale. Also W2 strip loads,
                and (on each expert's first chunk) the gated b2 y-init."""
                if c == 0:
                    for m in range(M2):
                        nc.vector.tensor_scalar_mul(
                            y_sb[:, m, :], wgg_sb[e][:], b2t_sb[e][:, m : m + 1]
                        )
                hts = []
                for f in range(FC):
                    ht = ht_pool.tile(
                        [P, CAP], F32R, tag="ht", name=f"ht_{e}_{c}_{f}"
                    )
                    col = c * FC + f
                    for h, (off, hs) in enumerate(HS):
                        nc.scalar.activation(
                            ht[:, off : off + hs],
                            psums[f][h][:],
                            GELU,
                            bias=b1t_sb[e][:, col : col + 1],
                        )
                    nc.vector.tensor_mul(ht[:], ht[:], wgg_sb[e][:])
                    hts.append(ht)
                w2s = []
                for f in range(FC):
                    w2f = w2_pool.tile(
                        [P, D_MODEL], F32R, tag="w2s", name=f"w2s_{e}_{c}_{f}"
                    )
                    row = (c * FC + f) * P
                    nc.sync.dma_start(w2f[:], w2[e][row : row + P, :])
                    w2s.append(w2f)
                return hts, w2s

            def emit_l2(e, c, hts, w2s):
                """PE: layer-2 matmuls; DVE: accumulate into y; store at
                the expert's last chunk."""
                for m in range(M2):
                    for h, (off, hs) in enumerate(HS):
                        ps = psum_pool.tile(
                            [P, hs], F32, tag="ps", name=f"ps2_{e}_{c}_{m}_{h}"
                        )
                        for f in range(FC):
                            nc.tensor.matmul(
                                ps[:],
                                w2s[f][:, m * P : (m + 1) * P],
                                hts[f][:, off : off + hs],
                                start=(f == 0),
                                stop=(f == FC - 1),
                            )
                        ysl = y_sb[:, m, off : off + hs]
                        nc.vector.tensor_add(ysl, ysl, ps[:])
                    if c == NCHUNK - 1:
                        nc.sync.dma_start(yt3[e][:, m, :], y_sb[:, m, :])

            psums_cur = emit_l1(*pairs[0], first=True)
            for i, (e, c) in enumerate(pairs):
                hts, w2s = emit_act(e, c, psums_cur)
                if i + 1 < len(pairs):
                    psums_cur = emit_l1(*pairs[i + 1])
                emit_l2(e, c, hts, w2s)

    nc.compile()
    return nc


def _build_dense(nc):
    """Dense fallback: both experts over all tokens, gate-weighted."""
    HS = _blocks(TOKC)
    xt = nc.dram_tensor("xt", (D_MODEL, TOKC), F32R, kind="ExternalInput").ap()
    w1 = [
        nc.dram_tensor(f"w1_{e}", (D_MODEL, D_FF), F32R, kind="ExternalInput").ap()
        for e in range(2)
    ]
    w2 = [
        nc.dram_tensor(f"w2_{e}", (D_FF, D_MODEL), F32R, kind="ExternalInput").ap()
        for e in range(2)
    ]
    b1t = [
        nc.dram_tensor(f"b1t_{e}", (P, D_FF // P), F32, kind="ExternalInput").ap()
        for e in range(2)
    ]
    b2t = [
        nc.dram_tensor(f"b2t_{e}", (P, M2), F32, kind="ExternalInput").ap()
        for e in range(2)
    ]
    wg = [
        nc.dram_tensor(f"wg{e}", (P, TOKC), F32, kind="ExternalInput").ap()
        for e in range(2)
    ]
    yt = nc.dram_tensor("yt", (D_MODEL, TOKC), F32, kind="ExternalOutput").ap()

    with tile.TileContext(nc) as tc:
        with (
            tc.tile_pool(name="const", bufs=1) as const_pool,
            tc.tile_pool(name="w1s", bufs=5) as w1_pool,
            tc.tile_pool(name="w2s", bufs=5) as w2_pool,
            tc.tile_pool(name="ht", bufs=5) as ht_pool,
            tc.tile_pool(name="ps", bufs=8, space="PSUM") as psum_pool,
        ):
            xt_sb = const_pool.tile([P, KM, TOKC], F32R, tag="xt", name="xt_sb")
            y_sb = const_pool.tile([P, M2, TOKC], F32, tag="y", name="y_sb")
            wg_sb = [
                const_pool.tile([P, TOKC], F32, tag=f"wg{e}", name=f"wg{e}_sb")
                for e in range(2)
            ]
            b1t_sb = [
                const_pool.tile([P, D_FF // P], F32, tag=f"b1t{e}", name=f"b1t{e}_sb")
                for e in range(2)
            ]
            b2t_sb = [
                const_pool.tile([P, M2], F32, tag=f"b2t{e}", name=f"b2t{e}_sb")
                for e in range(2)
            ]

            xt3 = xt.rearrange("(ko p) t -> p ko t", p=P)
            pairs = [(e, c) for e in range(2) for c in range(NCHUNK)]

            def emit_l1(e, c, first=False):
                psums = [
                    [
                        psum_pool.tile(
                            [P, hs], F32, tag="ps", name=f"ps1_{e}_{c}_{f}_{h}"
                        )
                        for h, (off, hs) in enumerate(HS)
                    ]
                    for f in range(FC)
                ]
                for k in range(KM):
                    if first:
                        nc.sync.dma_start(xt_sb[:, k, :], xt3[:, k, :])
                        if k == 0:
                            for ee in range(2):
                                nc.sync.dma_start(wg_sb[ee][:], wg[ee][:])
                                nc.sync.dma_start(b1t_sb[ee][:], b1t[ee][:])
                                nc.sync.dma_start(b2t_sb[ee][:], b2t[ee][:])
                    w1s = w1_pool.tile(
                        [P, CHUNK], F32R, tag="w1s", name=f"w1s_{e}_{c}_{k}"
                    )
                    nc.sync.dma_start(
                        w1s[:],
                        w1[e][k * P : (k + 1) * P, c * CHUNK : (c + 1) * CHUNK],
                    )
                    for f in range(FC):
                        for h, (off, hs) in enumerate(HS):
                            nc.tensor.matmul(
                                psums[f][h][:],
                                w1s[:, f * P : (f + 1) * P],
                                xt_sb[:, k, off : off + hs],
                                start=(k == 0),
                                stop=(k == KM - 1),
                            )
                return psums

            def emit_act(e, c, psums):
                hts = []
                for f in range(FC):
                    ht = ht_pool.tile(
                        [P, TOKC], F32R, tag="ht", name=f"ht_{e}_{c}_{f}"
                    )
                    col = c * FC + f
                    for h, (off, hs) in enumerate(HS):
                        nc.scalar.activation(
                            ht[:, off : off + hs],
                            psums[f][h][:],
                            GELU,
                            bias=b1t_sb[e][:, col : col + 1],
                        )
                    nc.vector.tensor_mul(ht[:], ht[:], wg_sb[e][:])
                    hts.append(ht)
                w2s = []
                for f in range(FC):
                    w2f = w2_pool.tile(
                        [P, D_MODEL], F32R, tag="w2s", name=f"w2s_{e}_{c}_{f}"
                    )
                    row = (c * FC + f) * P
                    nc.sync.dma_start(w2f[:], w2[e][row : row + P, :])
                    w2s.append(w2f)
                return hts, w2s

            def emit_l2(e, c, hts, w2s):
                for m in range(M2):
                    for h, (off, hs) in enumerate(HS):
                        ps = psum_pool.tile(
                            [P, hs], F32, tag="ps", name=f"ps2_{e}_{c}_{m}_{h}"
                        )
                        for f in range(FC):
                            nc.tensor.matmul(
                                ps[:],
                                w2s[f][:, m * P : (m + 1) * P],
                                hts[f][:, off : off + hs],
                                start=(f == 0),
                                stop=(f == FC - 1),
                            )
                        ysl = y_sb[:, m, off : off + hs]
                        nc.vector.tensor_add(ysl, ysl, ps[:])

            psums_cur = emit_l1(*pairs[0], first=True)

            for m in range(M2):
                nc.vector.tensor_scalar_mul(
                    y_sb[:, m, :], wg_sb[0][:], b2t_sb[0][:, m : m + 1]
                )
                t = ht_pool.tile([P, TOKC], F32, tag="ht", name="ytmp")
                nc.vector.tensor_scalar_mul(
                    t[:], wg_sb[1][:], b2t_sb[1][:, m : m + 1]
                )
                nc.vector.tensor_add(y_sb[:, m, :], y_sb[:, m, :], t[:])

            for i, (e, c) in enumerate(pairs):
                hts, w2s = emit_act(e, c, psums_cur)
                if i + 1 < len(pairs):
                    psums_cur = emit_l1(*pairs[i + 1])
                emit_l2(e, c, hts, w2s)

            yt3 = yt.rearrange("(mo p) t -> p mo t", p=P)
            for m in range(M2):
                nc.sync.dma_start(yt3[:, m, :], y_sb[:, m, :])

    nc.compile()
    return nc


_CACHED = {}


def _get_nc(kind):
    if kind not in _CACHED:
        nc = bacc.Bacc(
            "TRN2",
            target_bir_lowering=False,
            debug=False,
            num_devices=N_CORES,
        )
        _CACHED[kind] = (_build_sparse if kind == "sparse" else _build_dense)(nc)
    return _CACHED[kind]


def _run(nc, in_maps):
    trace = bool(int(os.environ.get("KERNEL_TRACE", "0")))
    if trace:
        _ensure_ntff_hook()
    res = bass_utils.run_bass_kernel_spmd(
        nc, in_maps, core_ids=list(range(N_CORES)), trace=trace
    )
    if trace:
        kernel.last_exec_time_ns = res.exec_time_ns
        kernel.last_results = res
    return res


def kernel(**inputs):
    x = np.asarray(inputs["x_local"], dtype=np.float32)          # (8192, 2048)
    ids = np.asarray(inputs["top2_exp_id"])                       # (8192, 2)
    tw = np.asarray(inputs["top2_weight"], dtype=np.float32)      # (8192, 2)

    sel = (ids % 2).astype(np.float32)
    wge = [
        (tw * (1.0 - sel)).sum(axis=1).astype(np.float32),        # expert-0 gate
        (tw * sel).sum(axis=1).astype(np.float32),                # expert-1 gate
    ]

    xt = np.ascontiguousarray(x.T)                                # (2048, 8192)

    shared = {}
    for e in range(2):
        shared[f"w1_{e}"] = np.ascontiguousarray(
            np.asarray(inputs[f"W1_{e}"], dtype=np.float32)
        )
        shared[f"w2_{e}"] = np.ascontiguousarray(
            np.asarray(inputs[f"W2_{e}"], dtype=np.float32)
        )
        shared[f"b1t_{e}"] = np.ascontiguousarray(
            np.asarray(inputs[f"b1_{e}"], dtype=np.float32).reshape(D_FF // P, P).T
        )
        shared[f"b2t_{e}"] = np.ascontiguousarray(
            np.asarray(inputs[f"b2_{e}"], dtype=np.float32).reshape(M2, P).T
        )

    # per-(core, expert) active-token gathers
    locs = [[None, None] for _ in range(N_CORES)]
    overflow = False
    for c in range(N_CORES):
        for e in range(2):
            loc = np.flatnonzero(wge[e][c * TOKC : (c + 1) * TOKC] > 0)
            locs[c][e] = loc
            if len(loc) > CAP:
                overflow = True

    if not overflow:
        in_maps = []
        for c in range(N_CORES):
            tok = slice(c * TOKC, (c + 1) * TOKC)
            m = dict(shared)
            for e in range(2):
                loc = locs[c][e]
                cnt = len(loc)
                xgc = np.zeros((D_MODEL, CAP), np.float32)
                xgc[:, :cnt] = xt[:, tok][:, loc]
                m[f"xg{e}"] = xgc
                wggc = np.zeros((CAP,), np.float32)
                wggc[:cnt] = wge[e][tok][loc]
                m[f"wgg{e}"] = np.ascontiguousarray(
                    np.broadcast_to(wggc, (P, CAP))
                )
            in_maps.append(m)

        res = _run(_get_nc("sparse"), in_maps)

        y = np.zeros((N_LOCAL, D_MODEL), np.float32)
        for c in range(N_CORES):
            for e in range(2):
                loc = locs[c][e]
                cnt = len(loc)
                y[c * TOKC + loc] += res.results[c][f"yt{e}"].T[:cnt]
        return y

    # dense fallback (vanishingly rare: a gather exceeded capacity)
    in_maps = []
    for c in range(N_CORES):
        tok = slice(c * TOKC, (c + 1) * TOKC)
        m = dict(shared)
        m["xt"] = np.ascontiguousarray(xt[:, tok])
        for e in range(2):
            m[f"wg{e}"] = np.ascontiguousarray(
                np.broadcast_to(wge[e][tok], (P, TOKC)).astype(np.float32)
            )
        in_maps.append(m)
    res = _run(_get_nc("dense"), in_maps)
    ytc = np.concatenate([r["yt"] for r in res.results], axis=1)  # (2048, 8192)
    return np.ascontiguousarray(ytc.T)


# revision 12
# speedup vs baseline: 1.3867x; 1.0638x over previous
"""MoE top-2 (2 experts) FFN kernel for TRN2, 8 NeuronCores.

Problem (hardcoded):
  x:   (8192, 2048) f32 tokens
  two expert FFNs: d_model=2048 -> d_ff=8192 (gelu exact) -> 2048
  out[i] = w0[i] * FFN0(x[i]) + w1[i] * FFN1(x[i])
  where w_e[i] = sum of top2_weight[i, s] over slots s with (top2_exp_id[i,s] % 2) == e

Strategy:
  - Host: fold top-2 gating into per-token scalars w0/w1; transpose x;
    gather each expert's active tokens (those with w_e > 0, ~75% of
    tokens) into a padded capacity of 832 per core -> 18.75% fewer FLOPs
    than dense. Dense fallback if a gather overflows capacity.
  - Data-parallel over tokens: each of 8 cores takes 1024 tokens.
  - On-core: activations kept transposed ([d_model|d_ff on partitions] x
    [tokens on free dim]) so both matmul layers contract along partitions
    with weights in their natural HBM layout.
  - fp32r (FP22) matmuls with moving dim >= 256 -> full 1-cycle/row PE.
  - d_ff processed in chunks of 512; layer-2 partials accumulated into an
    SBUF-resident y so each weight byte is streamed exactly once.
  - Software-pipelined emission: PE order L1(0),L1(1),L2(0),L1(2),L2(1)...
    so gelu/gate (ACT+DVE) of chunk i overlaps L1(i+1) matmuls.
"""

import os

import numpy as np

import concourse.bass as bass
import concourse.mybir as mybir
import concourse.tile as tile
from concourse import bacc
from concourse import bass_utils


def _ensure_ntff_hook():
    """This image's `antenv` lacks `axon_hooks`, so boot-time NTFF hook
    install degrades silently and trace=True captures nothing. Register a
    shim module and install the ctypes-driven hook (same as trn_boot)."""
    import sys
    import types

    if "antenv.axon_hooks" in sys.modules:
        return
    mod = types.ModuleType("antenv.axon_hooks")
    mod._hook = None

    def set_axon_ntff_profile_hook(h):
        mod._hook = h

    def get_axon_ntff_profile_hook():
        return mod._hook

    mod.set_axon_ntff_profile_hook = set_axon_ntff_profile_hook
    mod.get_axon_ntff_profile_hook = get_axon_ntff_profile_hook
    sys.modules["antenv.axon_hooks"] = mod
    try:
        from trn_agent_boot.trn_boot import _ntff_profile_via_ctypes

        hook = _ntff_profile_via_ctypes("/opt/axon/libaxon_pjrt.so")
        if hook is not None:
            mod._hook = hook
    except Exception:
        pass


P = 128
D_MODEL = 2048
D_FF = 8192
N_LOCAL = 8192
N_CORES = 8
TOKC = N_LOCAL // N_CORES      # 1024 tokens per core
CAP = 792                      # per-expert gathered-token capacity per core
KM = D_MODEL // P              # 16 contraction tiles for layer 1
CHUNK = 512                    # d_ff chunk held in PSUM per pass
FC = CHUNK // P                # 4 d_ff tiles per chunk
NCHUNK = D_FF // CHUNK         # 16
M2 = D_MODEL // P              # 16 output d_model tiles

F32 = mybir.dt.float32
F32R = mybir.dt.float32r
GELU = mybir.ActivationFunctionType.Gelu


def _blocks(total):
    """Moving-dim blocks: each <= 512 (fp32 max) and >= 256 (fp32r full
    speed needs ap_size >= 256). fp32r matmuls are LDWEIGHTS-bound below
    N ~ 400, so equal blocks beat greedy 512-first splits."""
    n = (total + 511) // 512
    base = total // n
    out = []
    off = 0
    for i in range(n):
        hs = base + (1 if i < total - base * n else 0)
        out.append((off, hs))
        off += hs
    assert off == total and all(256 <= hs <= 512 for _, hs in out)
    return out


def _build_sparse(nc):
    """Per-expert gathered tokens (CAP per core); expert passes run
    back-to-back reusing the same xt/y SBUF buffers."""
    HS = _blocks(CAP)
    xg = [
        nc.dram_tensor(f"xg{e}", (D_MODEL, CAP), F32R, kind="ExternalInput").ap()
        for e in range(2)
    ]
    w1 = [
        nc.dram_tensor(f"w1_{e}", (D_MODEL, D_FF), F32R, kind="ExternalInput").ap()
        for e in range(2)
    ]
    w2 = [
        nc.dram_tensor(f"w2_{e}", (D_FF, D_MODEL), F32R, kind="ExternalInput").ap()
        for e in range(2)
    ]
    b1t = [
        nc.dram_tensor(f"b1t_{e}", (P, D_FF // P), F32, kind="ExternalInput").ap()
        for e in range(2)
    ]
    b2t = [
        nc.dram_tensor(f"b2t_{e}", (P, M2), F32, kind="ExternalInput").ap()
        for e in range(2)
    ]
    wgg = [
        nc.dram_tensor(f"wgg{e}", (P, CAP), F32, kind="ExternalInput").ap()
        for e in range(2)
    ]
    yt = [
        nc.dram_tensor(f"yt{e}", (D_MODEL, CAP), F32, kind="ExternalOutput").ap()
        for e in range(2)
    ]

    with tile.TileContext(nc) as tc:
        with (
            tc.tile_pool(name="const", bufs=1) as const_pool,
            tc.tile_pool(name="w1s", bufs=6) as w1_pool,
            tc.tile_pool(name="w2s", bufs=6) as w2_pool,
            tc.tile_pool(name="ht", bufs=6) as ht_pool,
            tc.tile_pool(name="ps", bufs=8, space="PSUM") as psum_pool,
        ):
            # xt k-tiles are separate so the second expert's reload can
            # start as soon as the first expert's last read of each tile
            # retires (fine-grained WAR deps).
            xt_sb = [
                const_pool.tile([P, CAP], F32R, tag=f"xt{k}", name=f"xt_sb{k}")
                for k in range(KM)
            ]
            y_sb = const_pool.tile([P, M2, CAP], F32, tag="y", name="y_sb")
            wgg_sb = [
                const_pool.tile([P, CAP], F32, tag=f"wgg{e}", name=f"wgg{e}_sb")
                for e in range(2)
            ]
            b1t_sb = [
                const_pool.tile([P, D_FF // P], F32, tag=f"b1t{e}", name=f"b1t{e}_sb")
                for e in range(2)
            ]
            b2t_sb = [
                const_pool.tile([P, M2], F32, tag=f"b2t{e}", name=f"b2t{e}_sb")
                for e in range(2)
            ]

            xg3 = [xg[e].rearrange("(ko p) t -> p ko t", p=P) for e in range(2)]
            yt3 = [yt[e].rearrange("(mo p) t -> p mo t", p=P) for e in range(2)]

            pairs = [(e, c) for e in range(2) for c in range(NCHUNK)]

            def emit_l1(e, c, first=False):
                """PE: layer-1 matmuls for one (expert, chunk)."""
                psums = [
                    [
                        psum_pool.tile(
                            [P, hs], F32, tag="ps", name=f"ps1_{e}_{c}_{f}_{h}"
                        )
                        for h, (off, hs) in enumerate(HS)
                    ]
                    for f in range(FC)
                ]
                for k in range(KM):
                    if c == 0:
                        # (re)load this expert's gathered xT, interleaved
                        # with the first chunk's weight strips
                        nc.sync.dma_start(xt_sb[k][:], xg3[e][:, k, :])
                        if first and k == 0:
                            for ee in range(2):
                                nc.sync.dma_start(wgg_sb[ee][:], wgg[ee][:])
                                nc.sync.dma_start(b1t_sb[ee][:], b1t[ee][:])
                                nc.sync.dma_start(b2t_sb[ee][:], b2t[ee][:])
                    w1s = w1_pool.tile(
                        [P, CHUNK], F32R, tag="w1s", name=f"w1s_{e}_{c}_{k}"
                    )
                    nc.sync.dma_start(
                        w1s[:],
                        w1[e][k * P : (k + 1) * P, c * CHUNK : (c + 1) * CHUNK],
                    )
                    for f in range(FC):
                        for h, (off, hs) in enumerate(HS):
                            nc.tensor.matmul(
                                psums[f][h][:],
                                w1s[:, f * P : (f + 1) * P],
                                xt_sb[k][:, off : off + hs],
                                start=(k == 0),
                                stop=(k == KM - 1),
                            )
                return psums

            def emit_act(e, c, psums):
                """ACT+DVE: gelu(+b1), gate scale. Also W2 strip loads,
                and (on each expert's first chunk) the gated b2 y-init."""
                if c == 0:
                    for m in range(M2):
                        nc.vector.tensor_scalar_mul(
                            y_sb[:, m, :], wgg_sb[e][:], b2t_sb[e][:, m : m + 1]
                        )
                hts = []
                for f in range(FC):
                    ht = ht_pool.tile(
                        [P, CAP], F32R, tag="ht", name=f"ht_{e}_{c}_{f}"
                    )
                    col = c * FC + f
                    for h, (off, hs) in enumerate(HS):
                        nc.scalar.activation(
                            ht[:, off : off + hs],
                            psums[f][h][:],
                            GELU,
                            bias=b1t_sb[e][:, col : col + 1],
                        )
                    nc.vector.tensor_mul(ht[:], ht[:], wgg_sb[e][:])
                    hts.append(ht)
                w2s = []
                for f in range(FC):
                    w2f = w2_pool.tile(
                        [P, D_MODEL], F32R, tag="w2s", name=f"w2s_{e}_{c}_{f}"
                    )
                    row = (c * FC + f) * P
                    nc.sync.dma_start(w2f[:], w2[e][row : row + P, :])
                    w2s.append(w2f)
                return hts, w2s

            def emit_l2(e, c, hts, w2s):
                """PE: layer-2 matmuls; DVE: accumulate into y; store at
                the expert's last chunk."""
                for m in range(M2):
                    for h, (off, hs) in enumerate(HS):
                        ps = psum_pool.tile(
                            [P, hs], F32, tag="ps", name=f"ps2_{e}_{c}_{m}_{h}"
                        )
                        for f in range(FC):
                            nc.tensor.matmul(
                                ps[:],
                                w2s[f][:, m * P : (m + 1) * P],
                                hts[f][:, off : off + hs],
                                start=(f == 0),
                                stop=(f == FC - 1),
                            )
                        ysl = y_sb[:, m, off : off + hs]
                        nc.vector.tensor_add(ysl, ysl, ps[:])
                    if c == NCHUNK - 1:
                        nc.sync.dma_start(yt3[e][:, m, :], y_sb[:, m, :])

            psums_cur = emit_l1(*pairs[0], first=True)
            for i, (e, c) in enumerate(pairs):
                hts, w2s = emit_act(e, c, psums_cur)
                if i + 1 < len(pairs):
                    psums_cur = emit_l1(*pairs[i + 1])
                emit_l2(e, c, hts, w2s)

    nc.compile()
    return nc


def _build_dense(nc):
    """Dense fallback: both experts over all tokens, gate-weighted."""
    HS = _blocks(TOKC)
    xt = nc.dram_tensor("xt", (D_MODEL, TOKC), F32R, kind="ExternalInput").ap()
    w1 = [
        nc.dram_tensor(f"w1_{e}", (D_MODEL, D_FF), F32R, kind="ExternalInput").ap()
        for e in range(2)
    ]
    w2 = [
        nc.dram_tensor(f"w2_{e}", (D_FF, D_MODEL), F32R, kind="ExternalInput").ap()
        for e in range(2)
    ]
    b1t = [
        nc.dram_tensor(f"b1t_{e}", (P, D_FF // P), F32, kind="ExternalInput").ap()
        for e in range(2)
    ]
    b2t = [
        nc.dram_tensor(f"b2t_{e}", (P, M2), F32, kind="ExternalInput").ap()
        for e in range(2)
    ]
    wg = [
        nc.dram_tensor(f"wg{e}", (P, TOKC), F32, kind="ExternalInput").ap()
        for e in range(2)
    ]
    yt = nc.dram_tensor("yt", (D_MODEL, TOKC), F32, kind="ExternalOutput").ap()

    with tile.TileContext(nc) as tc:
        with (
            tc.tile_pool(name="const", bufs=1) as const_pool,
            tc.tile_pool(name="w1s", bufs=5) as w1_pool,
            tc.tile_pool(name="w2s", bufs=5) as w2_pool,
            tc.tile_pool(name="ht", bufs=5) as ht_pool,
            tc.tile_pool(name="ps", bufs=8, space="PSUM") as psum_pool,
        ):
            xt_sb = const_pool.tile([P, KM, TOKC], F32R, tag="xt", name="xt_sb")
            y_sb = const_pool.tile([P, M2, TOKC], F32, tag="y", name="y_sb")
            wg_sb = [
                const_pool.tile([P, TOKC], F32, tag=f"wg{e}", name=f"wg{e}_sb")
                for e in range(2)
            ]
            b1t_sb = [
                const_pool.tile([P, D_FF // P], F32, tag=f"b1t{e}", name=f"b1t{e}_sb")
                for e in range(2)
            ]
            b2t_sb = [
                const_pool.tile([P, M2], F32, tag=f"b2t{e}", name=f"b2t{e}_sb")
                for e in range(2)
            ]

            xt3 = xt.rearrange("(ko p) t -> p ko t", p=P)
            pairs = [(e, c) for e in range(2) for c in range(NCHUNK)]

            def emit_l1(e, c, first=False):
                psums = [
                    [
                        psum_pool.tile(
                            [P, hs], F32, tag="ps", name=f"ps1_{e}_{c}_{f}_{h}"
                        )
                        for h, (off, hs) in enumerate(HS)
                    ]
                    for f in range(FC)
                ]
                for k in range(KM):
                    if first:
                        nc.sync.dma_start(xt_sb[:, k, :], xt3[:, k, :])
                        if k == 0:
                            for ee in range(2):
                                nc.sync.dma_start(wg_sb[ee][:], wg[ee][:])
                                nc.sync.dma_start(b1t_sb[ee][:], b1t[ee][:])
                                nc.sync.dma_start(b2t_sb[ee][:], b2t[ee][:])
                    w1s = w1_pool.tile(
                        [P, CHUNK], F32R, tag="w1s", name=f"w1s_{e}_{c}_{k}"
                    )
                    nc.sync.dma_start(
                        w1s[:],
                        w1[e][k * P : (k + 1) * P, c * CHUNK : (c + 1) * CHUNK],
                    )
                    for f in range(FC):
                        for h, (off, hs) in enumerate(HS):
                            nc.tensor.matmul(
                                psums[f][h][:],
                                w1s[:, f * P : (f + 1) * P],
                                xt_sb[:, k, off : off + hs],
                                start=(k == 0),
                                stop=(k == KM - 1),
                            )
                return psums

            def emit_act(e, c, psums):
                hts = []
                for f in range(FC):
                    ht = ht_pool.tile(
                        [P, TOKC], F32R, tag="ht", name=f"ht_{e}_{c}_{f}"
                    )
                    col = c * FC + f
                    for h, (off, hs) in enumerate(HS):
                        nc.scalar.activation(
                            ht[:, off : off + hs],
                            psums[f][h][:],
                            GELU,
                            bias=b1t_sb[e][:, col : col + 1],
                        )
                    nc.vector.tensor_mul(ht[:], ht[:], wg_sb[e][:])
                    hts.append(ht)
                w2s = []
                for f in range(FC):
                    w2f = w2_pool.tile(
                        [P, D_MODEL], F32R, tag="w2s", name=f"w2s_{e}_{c}_{f}"
                    )
                    row = (c * FC + f) * P
                    nc.sync.dma_start(w2f[:], w2[e][row : row + P, :])
                    w2s.append(w2f)
                return hts, w2s

            def emit_l2(e, c, hts, w2s):
                for m in range(M2):
                    for h, (off, hs) in enumerate(HS):
                        ps = psum_pool.tile(
                            [P, hs], F32, tag="ps", name=f"ps2_{e}_{c}_{m}_{h}"
                        )
                        for f in range(FC):
                            nc.tensor.matmul(
                                ps[:],
                                w2s[f][:, m * P : (m + 1) * P],
                                hts[f][:, off : off + hs],
                                start=(f == 0),
                                stop=(f == FC - 1),
                            )
                        ysl = y_sb[:, m, off : off + hs]
                        nc.vector.tensor_add(ysl, ysl, ps[:])

            psums_cur = emit_l1(*pairs[0], first=True)

            for m in range(M2):
                nc.vector.tensor_scalar_mul(
                    y_sb[:, m, :], wg_sb[0][:], b2t_sb[0][:, m : m + 1]
                )
                t = ht_pool.tile([P, TOKC], F32, tag="ht", name="ytmp")
                nc.vector.tensor_scalar_mul(
                    t[:], wg_sb[1][:], b2t_sb[1][:, m : m + 1]
                )
                nc.vector.tensor_add(y_sb[:, m, :], y_sb[:, m, :], t[:])

            for i, (e, c) in enumerate(pairs):
                hts, w2s = emit_act(e, c, psums_cur)
                if i + 1 < len(pairs):
                    psums_cur = emit_l1(*pairs[i + 1])
                emit_l2(e, c, hts, w2s)

            yt3 = yt.rearrange("(mo p) t -> p mo t", p=P)
            for m in range(M2):
                nc.sync.dma_start(yt3[:, m, :], y_sb[:, m, :])

    nc.compile()
    return nc


_CACHED = {}


def _get_nc(kind):
    if kind not in _CACHED:
        nc = bacc.Bacc(
            "TRN2",
            target_bir_lowering=False,
            debug=False,
            num_devices=N_CORES,
        )
        _CACHED[kind] = (_build_sparse if kind == "sparse" else _build_dense)(nc)
    return _CACHED[kind]


def _run(nc, in_maps):
    trace = bool(int(os.environ.get("KERNEL_TRACE", "0")))
    if trace:
        _ensure_ntff_hook()
    res = bass_utils.run_bass_kernel_spmd(
        nc, in_maps, core_ids=list(range(N_CORES)), trace=trace
    )
    if trace:
        kernel.last_exec_time_ns = res.exec_time_ns
        kernel.last_results = res
    return res


def kernel(**inputs):
    x = np.asarray(inputs["x_local"], dtype=np.float32)          # (8192, 2048)
    ids = np.asarray(inputs["top2_exp_id"])                       # (8192, 2)
    tw = np.asarray(inputs["top2_weight"], dtype=np.float32)      # (8192, 2)

    sel = (ids % 2).astype(np.float32)
    wge = [
        (tw * (1.0 - sel)).sum(axis=1).astype(np.float32),        # expert-0 gate
        (tw * sel).sum(axis=1).astype(np.float32),                # expert-1 gate
    ]

    xt = np.ascontiguousarray(x.T)                                # (2048, 8192)

    shared = {}
    for e in range(2):
        shared[f"w1_{e}"] = np.ascontiguousarray(
            np.asarray(inputs[f"W1_{e}"], dtype=np.float32)
        )
        shared[f"w2_{e}"] = np.ascontiguousarray(
            np.asarray(inputs[f"W2_{e}"], dtype=np.float32)
        )
        shared[f"b1t_{e}"] = np.ascontiguousarray(
            np.asarray(inputs[f"b1_{e}"], dtype=np.float32).reshape(D_FF // P, P).T
        )
        shared[f"b2t_{e}"] = np.ascontiguousarray(
            np.asarray(inputs[f"b2_{e}"], dtype=np.float32).reshape(M2, P).T
        )

    # Globally-balanced gathers: each expert's active set (~75% of all
    # tokens) is split evenly across the 8 cores, so per-core load is
    # |S_e|/8 +- 1 regardless of which core a token "belongs" to.
    glocs = [np.flatnonzero(wge[e] > 0) for e in range(2)]
    overflow = any(len(g) > CAP * N_CORES for g in glocs)

    if not overflow:
        splits = [np.array_split(glocs[e], N_CORES) for e in range(2)]
        in_maps = []
        for c in range(N_CORES):
            m = dict(shared)
            for e in range(2):
                loc = splits[e][c]
                cnt = len(loc)
                xgc = np.zeros((D_MODEL, CAP), np.float32)
                xgc[:, :cnt] = xt[:, loc]
                m[f"xg{e}"] = xgc
                wggc = np.zeros((CAP,), np.float32)
                wggc[:cnt] = wge[e][loc]
                m[f"wgg{e}"] = np.ascontiguousarray(
                    np.broadcast_to(wggc, (P, CAP))
                )
            in_maps.append(m)

        res = _run(_get_nc("sparse"), in_maps)

        y = np.zeros((N_LOCAL, D_MODEL), np.float32)
        for c in range(N_CORES):
            for e in range(2):
                loc = splits[e][c]
                cnt = len(loc)
                y[loc] += res.results[c][f"yt{e}"].T[:cnt]
        return y

    # dense fallback (vanishingly rare: a gather exceeded capacity)
    in_maps = []
    for c in range(N_CORES):
        tok = slice(c * TOKC, (c + 1) * TOKC)
        m = dict(shared)
        m["xt"] = np.ascontiguousarray(xt[:, tok])
        for e in range(2):
            m[f"wg{e}"] = np.ascontiguousarray(
                np.broadcast_to(wge[e][tok], (P, TOKC)).astype(np.float32)
            )
        in_maps.append(m)
    res = _run(_get_nc("dense"), in_maps)
    ytc = np.concatenate([r["yt"] for r in res.results], axis=1)  # (2048, 8192)
    return np.ascontiguousarray(ytc.T)


# revision 13
# speedup vs baseline: 1.4110x; 1.0175x over previous
"""MoE top-2 (2 experts) FFN kernel for TRN2, 8 NeuronCores.

Problem (hardcoded):
  x:   (8192, 2048) f32 tokens
  two expert FFNs: d_model=2048 -> d_ff=8192 (gelu exact) -> 2048
  out[i] = w0[i] * FFN0(x[i]) + w1[i] * FFN1(x[i])
  where w_e[i] = sum of top2_weight[i, s] over slots s with (top2_exp_id[i,s] % 2) == e

Strategy:
  - Host: fold top-2 gating into per-token scalars w0/w1; transpose x;
    gather each expert's active tokens (those with w_e > 0, ~75% of
    tokens) into a padded capacity of 832 per core -> 18.75% fewer FLOPs
    than dense. Dense fallback if a gather overflows capacity.
  - Data-parallel over tokens: each of 8 cores takes 1024 tokens.
  - On-core: activations kept transposed ([d_model|d_ff on partitions] x
    [tokens on free dim]) so both matmul layers contract along partitions
    with weights in their natural HBM layout.
  - fp32r (FP22) matmuls with moving dim >= 256 -> full 1-cycle/row PE.
  - d_ff processed in chunks of 512; layer-2 partials accumulated into an
    SBUF-resident y so each weight byte is streamed exactly once.
  - Software-pipelined emission: PE order L1(0),L1(1),L2(0),L1(2),L2(1)...
    so gelu/gate (ACT+DVE) of chunk i overlaps L1(i+1) matmuls.
"""

import os

import numpy as np

import concourse.bass as bass
import concourse.mybir as mybir
import concourse.tile as tile
from concourse import bacc
from concourse import bass_utils


def _ensure_ntff_hook():
    """This image's `antenv` lacks `axon_hooks`, so boot-time NTFF hook
    install degrades silently and trace=True captures nothing. Register a
    shim module and install the ctypes-driven hook (same as trn_boot)."""
    import sys
    import types

    if "antenv.axon_hooks" in sys.modules:
        return
    mod = types.ModuleType("antenv.axon_hooks")
    mod._hook = None

    def set_axon_ntff_profile_hook(h):
        mod._hook = h

    def get_axon_ntff_profile_hook():
        return mod._hook

    mod.set_axon_ntff_profile_hook = set_axon_ntff_profile_hook
    mod.get_axon_ntff_profile_hook = get_axon_ntff_profile_hook
    sys.modules["antenv.axon_hooks"] = mod
    try:
        from trn_agent_boot.trn_boot import _ntff_profile_via_ctypes

        hook = _ntff_profile_via_ctypes("/opt/axon/libaxon_pjrt.so")
        if hook is not None:
            mod._hook = hook
    except Exception:
        pass


P = 128
D_MODEL = 2048
D_FF = 8192
N_LOCAL = 8192
N_CORES = 8
TOKC = N_LOCAL // N_CORES      # 1024 tokens per core
CAP = 784                      # per-expert gathered-token capacity per core
KM = D_MODEL // P              # 16 contraction tiles for layer 1
CHUNK = 512                    # d_ff chunk held in PSUM per pass
FC = CHUNK // P                # 4 d_ff tiles per chunk
NCHUNK = D_FF // CHUNK         # 16
M2 = D_MODEL // P              # 16 output d_model tiles

F32 = mybir.dt.float32
F32R = mybir.dt.float32r
GELU = mybir.ActivationFunctionType.Gelu


def _blocks(total):
    """Moving-dim blocks: each <= 512 (fp32 max) and >= 256 (fp32r full
    speed needs ap_size >= 256). fp32r matmuls are LDWEIGHTS-bound below
    N ~ 400, so equal blocks beat greedy 512-first splits."""
    n = (total + 511) // 512
    base = total // n
    out = []
    off = 0
    for i in range(n):
        hs = base + (1 if i < total - base * n else 0)
        out.append((off, hs))
        off += hs
    assert off == total and all(256 <= hs <= 512 for _, hs in out)
    return out


def _build_sparse(nc):
    """Per-expert gathered tokens (CAP per core); expert passes run
    back-to-back reusing the same xt/y SBUF buffers."""
    HS = _blocks(CAP)
    xg = [
        nc.dram_tensor(f"xg{e}", (D_MODEL, CAP), F32R, kind="ExternalInput").ap()
        for e in range(2)
    ]
    w1 = [
        nc.dram_tensor(f"w1_{e}", (D_MODEL, D_FF), F32R, kind="ExternalInput").ap()
        for e in range(2)
    ]
    w2 = [
        nc.dram_tensor(f"w2_{e}", (D_FF, D_MODEL), F32R, kind="ExternalInput").ap()
        for e in range(2)
    ]
    b1t = [
        nc.dram_tensor(f"b1t_{e}", (P, D_FF // P), F32, kind="ExternalInput").ap()
        for e in range(2)
    ]
    b2t = [
        nc.dram_tensor(f"b2t_{e}", (P, M2), F32, kind="ExternalInput").ap()
        for e in range(2)
    ]
    wgg = [
        nc.dram_tensor(f"wgg{e}", (P, CAP), F32, kind="ExternalInput").ap()
        for e in range(2)
    ]
    yt = [
        nc.dram_tensor(f"yt{e}", (D_MODEL, CAP), F32, kind="ExternalOutput").ap()
        for e in range(2)
    ]

    with tile.TileContext(nc) as tc:
        with (
            tc.tile_pool(name="const", bufs=1) as const_pool,
            tc.tile_pool(name="w1s", bufs=8) as w1_pool,
            tc.tile_pool(name="w2s", bufs=6) as w2_pool,
            tc.tile_pool(name="ht", bufs=8) as ht_pool,
            tc.tile_pool(name="ps", bufs=8, space="PSUM") as psum_pool,
        ):
            # xt k-tiles are separate so the second expert's reload can
            # start as soon as the first expert's last read of each tile
            # retires (fine-grained WAR deps).
            xt_sb = [
                const_pool.tile([P, CAP], F32R, tag=f"xt{k}", name=f"xt_sb{k}")
                for k in range(KM)
            ]
            y_sb = const_pool.tile([P, M2, CAP], F32, tag="y", name="y_sb")
            wgg_sb = [
                const_pool.tile([P, CAP], F32, tag=f"wgg{e}", name=f"wgg{e}_sb")
                for e in range(2)
            ]
            b1t_sb = [
                const_pool.tile([P, D_FF // P], F32, tag=f"b1t{e}", name=f"b1t{e}_sb")
                for e in range(2)
            ]
            b2t_sb = [
                const_pool.tile([P, M2], F32, tag=f"b2t{e}", name=f"b2t{e}_sb")
                for e in range(2)
            ]

            xg3 = [xg[e].rearrange("(ko p) t -> p ko t", p=P) for e in range(2)]
            yt3 = [yt[e].rearrange("(mo p) t -> p mo t", p=P) for e in range(2)]

            pairs = [(e, c) for e in range(2) for c in range(NCHUNK)]

            def emit_l1(e, c, first=False):
                """PE: layer-1 matmuls for one (expert, chunk)."""
                psums = [
                    [
                        psum_pool.tile(
                            [P, hs], F32, tag="ps", name=f"ps1_{e}_{c}_{f}_{h}"
                        )
                        for h, (off, hs) in enumerate(HS)
                    ]
                    for f in range(FC)
                ]
                for k in range(KM):
                    if c == 0:
                        # (re)load this expert's gathered xT, interleaved
                        # with the first chunk's weight strips. The very
                        # first tiles gate the first matmul -> split them
                        # across several DMA queues.
                        nsplit = 4 if (first and k == 0) else 1
                        for s in range(nsplit):
                            sl = slice(s * CAP // nsplit, (s + 1) * CAP // nsplit)
                            nc.sync.dma_start(xt_sb[k][:, sl], xg3[e][:, k, sl])
                        if first and k == 0:
                            for ee in range(2):
                                nc.sync.dma_start(wgg_sb[ee][:], wgg[ee][:])
                                nc.sync.dma_start(b1t_sb[ee][:], b1t[ee][:])
                                nc.sync.dma_start(b2t_sb[ee][:], b2t[ee][:])
                    w1s = w1_pool.tile(
                        [P, CHUNK], F32R, tag="w1s", name=f"w1s_{e}_{c}_{k}"
                    )
                    nsplit = 4 if (first and c == 0 and k == 0) else 1
                    for s in range(nsplit):
                        sl = slice(s * CHUNK // nsplit, (s + 1) * CHUNK // nsplit)
                        nc.sync.dma_start(
                            w1s[:, sl],
                            w1[e][
                                k * P : (k + 1) * P,
                                c * CHUNK + sl.start : c * CHUNK + sl.stop,
                            ],
                        )
                    for f in range(FC):
                        for h, (off, hs) in enumerate(HS):
                            nc.tensor.matmul(
                                psums[f][h][:],
                                w1s[:, f * P : (f + 1) * P],
                                xt_sb[k][:, off : off + hs],
                                start=(k == 0),
                                stop=(k == KM - 1),
                            )
                return psums

            def emit_act(e, c, psums):
                """ACT+DVE: gelu(+b1), gate scale. Also W2 strip loads,
                and (on each expert's first chunk) the gated b2 y-init."""
                if c == 0:
                    for m in range(M2):
                        nc.vector.tensor_scalar_mul(
                            y_sb[:, m, :], wgg_sb[e][:], b2t_sb[e][:, m : m + 1]
                        )
                hts = []
                for f in range(FC):
                    ht = ht_pool.tile(
                        [P, CAP], F32R, tag="ht", name=f"ht_{e}_{c}_{f}"
                    )
                    col = c * FC + f
                    for h, (off, hs) in enumerate(HS):
                        nc.scalar.activation(
                            ht[:, off : off + hs],
                            psums[f][h][:],
                            GELU,
                            bias=b1t_sb[e][:, col : col + 1],
                        )
                    nc.vector.tensor_mul(ht[:], ht[:], wgg_sb[e][:])
                    hts.append(ht)
                w2s = []
                for f in range(FC):
                    w2f = w2_pool.tile(
                        [P, D_MODEL], F32R, tag="w2s", name=f"w2s_{e}_{c}_{f}"
                    )
                    row = (c * FC + f) * P
                    nc.sync.dma_start(w2f[:], w2[e][row : row + P, :])
                    w2s.append(w2f)
                return hts, w2s

            def emit_l2(e, c, hts, w2s):
                """PE: layer-2 matmuls; DVE: accumulate into y; store at
                the expert's last chunk."""
                for m in range(M2):
                    for h, (off, hs) in enumerate(HS):
                        ps = psum_pool.tile(
                            [P, hs], F32, tag="ps", name=f"ps2_{e}_{c}_{m}_{h}"
                        )
                        for f in range(FC):
                            nc.tensor.matmul(
                                ps[:],
                                w2s[f][:, m * P : (m + 1) * P],
                                hts[f][:, off : off + hs],
                                start=(f == 0),
                                stop=(f == FC - 1),
                            )
                        ysl = y_sb[:, m, off : off + hs]
                        nc.vector.tensor_add(ysl, ysl, ps[:])
                    if c == NCHUNK - 1:
                        nc.sync.dma_start(yt3[e][:, m, :], y_sb[:, m, :])

            psums_cur = emit_l1(*pairs[0], first=True)
            for i, (e, c) in enumerate(pairs):
                hts, w2s = emit_act(e, c, psums_cur)
                if i + 1 < len(pairs):
                    psums_cur = emit_l1(*pairs[i + 1])
                emit_l2(e, c, hts, w2s)

    nc.compile()
    return nc


def _build_dense(nc):
    """Dense fallback: both experts over all tokens, gate-weighted."""
    HS = _blocks(TOKC)
    xt = nc.dram_tensor("xt", (D_MODEL, TOKC), F32R, kind="ExternalInput").ap()
    w1 = [
        nc.dram_tensor(f"w1_{e}", (D_MODEL, D_FF), F32R, kind="ExternalInput").ap()
        for e in range(2)
    ]
    w2 = [
        nc.dram_tensor(f"w2_{e}", (D_FF, D_MODEL), F32R, kind="ExternalInput").ap()
        for e in range(2)
    ]
    b1t = [
        nc.dram_tensor(f"b1t_{e}", (P, D_FF // P), F32, kind="ExternalInput").ap()
        for e in range(2)
    ]
    b2t = [
        nc.dram_tensor(f"b2t_{e}", (P, M2), F32, kind="ExternalInput").ap()
        for e in range(2)
    ]
    wg = [
        nc.dram_tensor(f"wg{e}", (P, TOKC), F32, kind="ExternalInput").ap()
        for e in range(2)
    ]
    yt = nc.dram_tensor("yt", (D_MODEL, TOKC), F32, kind="ExternalOutput").ap()

    with tile.TileContext(nc) as tc:
        with (
            tc.tile_pool(name="const", bufs=1) as const_pool,
            tc.tile_pool(name="w1s", bufs=5) as w1_pool,
            tc.tile_pool(name="w2s", bufs=5) as w2_pool,
            tc.tile_pool(name="ht", bufs=5) as ht_pool,
            tc.tile_pool(name="ps", bufs=8, space="PSUM") as psum_pool,
        ):
            xt_sb = const_pool.tile([P, KM, TOKC], F32R, tag="xt", name="xt_sb")
            y_sb = const_pool.tile([P, M2, TOKC], F32, tag="y", name="y_sb")
            wg_sb = [
                const_pool.tile([P, TOKC], F32, tag=f"wg{e}", name=f"wg{e}_sb")
                for e in range(2)
            ]
            b1t_sb = [
                const_pool.tile([P, D_FF // P], F32, tag=f"b1t{e}", name=f"b1t{e}_sb")
                for e in range(2)
            ]
            b2t_sb = [
                const_pool.tile([P, M2], F32, tag=f"b2t{e}", name=f"b2t{e}_sb")
                for e in range(2)
            ]

            xt3 = xt.rearrange("(ko p) t -> p ko t", p=P)
            pairs = [(e, c) for e in range(2) for c in range(NCHUNK)]

            def emit_l1(e, c, first=False):
                psums = [
                    [
                        psum_pool.tile(
                            [P, hs], F32, tag="ps", name=f"ps1_{e}_{c}_{f}_{h}"
                        )
                        for h, (off, hs) in enumerate(HS)
                    ]
                    for f in range(FC)
                ]
                for k in range(KM):
                    if first:
                        nc.sync.dma_start(xt_sb[:, k, :], xt3[:, k, :])
                        if k == 0:
                            for ee in range(2):
                                nc.sync.dma_start(wg_sb[ee][:], wg[ee][:])
                                nc.sync.dma_start(b1t_sb[ee][:], b1t[ee][:])
                                nc.sync.dma_start(b2t_sb[ee][:], b2t[ee][:])
                    w1s = w1_pool.tile(
                        [P, CHUNK], F32R, tag="w1s", name=f"w1s_{e}_{c}_{k}"
                    )
                    nc.sync.dma_start(
                        w1s[:],
                        w1[e][k * P : (k + 1) * P, c * CHUNK : (c + 1) * CHUNK],
                    )
                    for f in range(FC):
                        for h, (off, hs) in enumerate(HS):
                            nc.tensor.matmul(
                                psums[f][h][:],
                                w1s[:, f * P : (f + 1) * P],
                                xt_sb[:, k, off : off + hs],
                                start=(k == 0),
                                stop=(k == KM - 1),
                            )
                return psums

            def emit_act(e, c, psums):
                hts = []
                for f in range(FC):
                    ht = ht_pool.tile(
                        [P, TOKC], F32R, tag="ht", name=f"ht_{e}_{c}_{f}"
                    )
                    col = c * FC + f
                    for h, (off, hs) in enumerate(HS):
                        nc.scalar.activation(
                            ht[:, off : off + hs],
                            psums[f][h][:],
                            GELU,
                            bias=b1t_sb[e][:, col : col + 1],
                        )
                    nc.vector.tensor_mul(ht[:], ht[:], wg_sb[e][:])
                    hts.append(ht)
                w2s = []
                for f in range(FC):
                    w2f = w2_pool.tile(
                        [P, D_MODEL], F32R, tag="w2s", name=f"w2s_{e}_{c}_{f}"
                    )
                    row = (c * FC + f) * P
                    nc.sync.dma_start(w2f[:], w2[e][row : row + P, :])
                    w2s.append(w2f)
                return hts, w2s

            def emit_l2(e, c, hts, w2s):
                for m in range(M2):
                    for h, (off, hs) in enumerate(HS):
                        ps = psum_pool.tile(
                            [P, hs], F32, tag="ps", name=f"ps2_{e}_{c}_{m}_{h}"
                        )
                        for f in range(FC):
                            nc.tensor.matmul(
                                ps[:],
                                w2s[f][:, m * P : (m + 1) * P],
                                hts[f][:, off : off + hs],
                                start=(f == 0),
                                stop=(f == FC - 1),
                            )
                        ysl = y_sb[:, m, off : off + hs]
                        nc.vector.tensor_add(ysl, ysl, ps[:])

            psums_cur = emit_l1(*pairs[0], first=True)

            for m in range(M2):
                nc.vector.tensor_scalar_mul(
                    y_sb[:, m, :], wg_sb[0][:], b2t_sb[0][:, m : m + 1]
                )
                t = ht_pool.tile([P, TOKC], F32, tag="ht", name="ytmp")
                nc.vector.tensor_scalar_mul(
                    t[:], wg_sb[1][:], b2t_sb[1][:, m : m + 1]
                )
                nc.vector.tensor_add(y_sb[:, m, :], y_sb[:, m, :], t[:])

            for i, (e, c) in enumerate(pairs):
                hts, w2s = emit_act(e, c, psums_cur)
                if i + 1 < len(pairs):
                    psums_cur = emit_l1(*pairs[i + 1])
                emit_l2(e, c, hts, w2s)

            yt3 = yt.rearrange("(mo p) t -> p mo t", p=P)
            for m in range(M2):
                nc.sync.dma_start(yt3[:, m, :], y_sb[:, m, :])

    nc.compile()
    return nc


_CACHED = {}


def _get_nc(kind):
    if kind not in _CACHED:
        nc = bacc.Bacc(
            "TRN2",
            target_bir_lowering=False,
            debug=False,
            num_devices=N_CORES,
        )
        _CACHED[kind] = (_build_sparse if kind == "sparse" else _build_dense)(nc)
    return _CACHED[kind]


def _run(nc, in_maps):
    trace = bool(int(os.environ.get("KERNEL_TRACE", "0")))
    if trace:
        _ensure_ntff_hook()
    res = bass_utils.run_bass_kernel_spmd(
        nc, in_maps, core_ids=list(range(N_CORES)), trace=trace
    )
    if trace:
        kernel.last_exec_time_ns = res.exec_time_ns
        kernel.last_results = res
    return res


def kernel(**inputs):
    x = np.asarray(inputs["x_local"], dtype=np.float32)          # (8192, 2048)
    ids = np.asarray(inputs["top2_exp_id"])                       # (8192, 2)
    tw = np.asarray(inputs["top2_weight"], dtype=np.float32)      # (8192, 2)

    sel = (ids % 2).astype(np.float32)
    wge = [
        (tw * (1.0 - sel)).sum(axis=1).astype(np.float32),        # expert-0 gate
        (tw * sel).sum(axis=1).astype(np.float32),                # expert-1 gate
    ]

    xt = np.ascontiguousarray(x.T)                                # (2048, 8192)

    shared = {}
    for e in range(2):
        shared[f"w1_{e}"] = np.ascontiguousarray(
            np.asarray(inputs[f"W1_{e}"], dtype=np.float32)
        )
        shared[f"w2_{e}"] = np.ascontiguousarray(
            np.asarray(inputs[f"W2_{e}"], dtype=np.float32)
        )
        shared[f"b1t_{e}"] = np.ascontiguousarray(
            np.asarray(inputs[f"b1_{e}"], dtype=np.float32).reshape(D_FF // P, P).T
        )
        shared[f"b2t_{e}"] = np.ascontiguousarray(
            np.asarray(inputs[f"b2_{e}"], dtype=np.float32).reshape(M2, P).T
        )

    # Globally-balanced gathers: each expert's active set (~75% of all
    # tokens) is split evenly across the 8 cores, so per-core load is
    # |S_e|/8 +- 1 regardless of which core a token "belongs" to.
    glocs = [np.flatnonzero(wge[e] > 0) for e in range(2)]
    overflow = any(len(g) > CAP * N_CORES for g in glocs)

    if not overflow:
        splits = [np.array_split(glocs[e], N_CORES) for e in range(2)]
        in_maps = []
        for c in range(N_CORES):
            m = dict(shared)
            for e in range(2):
                loc = splits[e][c]
                cnt = len(loc)
                xgc = np.zeros((D_MODEL, CAP), np.float32)
                xgc[:, :cnt] = xt[:, loc]
                m[f"xg{e}"] = xgc
                wggc = np.zeros((CAP,), np.float32)
                wggc[:cnt] = wge[e][loc]
                m[f"wgg{e}"] = np.ascontiguousarray(
                    np.broadcast_to(wggc, (P, CAP))
                )
            in_maps.append(m)

        res = _run(_get_nc("sparse"), in_maps)

        y = np.zeros((N_LOCAL, D_MODEL), np.float32)
        for c in range(N_CORES):
            for e in range(2):
                loc = splits[e][c]
                cnt = len(loc)
                y[loc] += res.results[c][f"yt{e}"].T[:cnt]
        return y

    # dense fallback (vanishingly rare: a gather exceeded capacity)
    in_maps = []
    for c in range(N_CORES):
        tok = slice(c * TOKC, (c + 1) * TOKC)
        m = dict(shared)
        m["xt"] = np.ascontiguousarray(xt[:, tok])
        for e in range(2):
            m[f"wg{e}"] = np.ascontiguousarray(
                np.broadcast_to(wge[e][tok], (P, TOKC)).astype(np.float32)
            )
        in_maps.append(m)
    res = _run(_get_nc("dense"), in_maps)
    ytc = np.concatenate([r["yt"] for r in res.results], axis=1)  # (2048, 8192)
    return np.ascontiguousarray(ytc.T)
